# revision 4
# baseline (speedup 1.0000x reference)
"""Trainium2 Bass kernel for nn_Ca_Aware_Embedder (histogram distance binning + embed).

z[b, i, j, :] = W[:, bin(d_ij)] + b  where d_ij = ||x_i - x_j||^2 binned into 15
squared-distance buckets (or zeros when d falls below the first edge / on an edge).

Strategy (8 NeuronCores, row-parallel over i, no collectives):
  - d computed per core in [128 i, 1024 j] layout with the reference's exact fp32
    op order ((xj-xi)^2 sums), so binning is bit-exact vs the jax oracle.
  - per group of 8 i-rows: DMA-replicate each d row across 16 partitions so
    partitions = (i_lo, bin_k); two fused DVE compares (d > lo_k) - (d >= hi_k)
    build an exact {0,1} one-hot in bf16.
  - PE produces output tiles [128 j, 8 i x 128 c] via one-hot x block-diagonal W:
    a bf16 pass with W_hi = bf16(W) plus an accumulating fp16 pass with
    (W - W_hi) * 2^9 against one-hot * 2^-9 (exact products; ~2e-7 rel error).
  - PSUM -> SBUF copies split across DVE and ACT, then one 512 KB DMA per tile
    into the output (64 KB contiguous runs in HBM).
"""

import sys

if "/opt/trn_rl_repo" not in sys.path:
    sys.path.insert(0, "/opt/trn_rl_repo")

import numpy as np
import ml_dtypes

import concourse.bass as bass
import concourse.mybir as mybir
import concourse.tile as tile
from concourse import bacc, bass_utils
from concourse.alu_op_type import AluOpType

F32 = mybir.dt.float32
BF16 = mybir.dt.bfloat16
FP16 = mybir.dt.float16

N_RES = 1024
C_Z = 128
NO_BINS = 15
MIN_BIN = 3.25
MAX_BIN = 20.75
INF = 100000000.0
N_CORES = 8
ROWS_PER_CORE = N_RES // N_CORES  # 128
GROUPS = ROWS_PER_CORE // 8  # 16 groups of 8 i-rows
JTILES = N_RES // 128  # 8
BIG = np.float32(3.4028235e38)  # finite sentinel > any d
LO_SCALE = float(2.0 ** -9)

_PROGRAM = None  # (nc, names) cache — build once per process


def _sq_bins() -> np.ndarray:
    """Exact f32 squared bin edges, matching jnp.linspace(...)**2 on this stack."""
    import jax.numpy as jnp

    bins = jnp.linspace(MIN_BIN, MAX_BIN, NO_BINS, dtype=jnp.float32)
    return np.asarray(bins * bins, dtype=np.float32)


def _build_program():
    nc = bacc.Bacc(
        "TRN2",
        target_bir_lowering=False,
        debug=False,
        enable_asserts=False,
        num_devices=N_CORES,
    )

    xjb_d = nc.dram_tensor("xjb", [128, 3 * N_RES], F32, kind="ExternalInput")
    xi_d = nc.dram_tensor("xi", [128, 4], F32, kind="ExternalInput")
    locol_d = nc.dram_tensor("locol", [128, 1], F32, kind="ExternalInput")
    hicol_d = nc.dram_tensor("hicol", [128, 1], F32, kind="ExternalInput")
    whi_d = nc.dram_tensor("whi", [128, 1024], BF16, kind="ExternalInput")
    wlo_d = nc.dram_tensor("wlo", [128, 1024], FP16, kind="ExternalInput")
    out_d = nc.dram_tensor(
        "out", [ROWS_PER_CORE, N_RES, C_Z], F32, kind="ExternalOutput"
    )

    with tile.TileContext(nc) as tc:
        with (
            tc.tile_pool(name="const", bufs=1) as cpool,
            tc.tile_pool(name="dstore", bufs=1) as dpool,
            tc.tile_pool(name="work", bufs=2) as wpool,
            tc.tile_pool(name="ohp", bufs=2) as ohpool,
            tc.tile_pool(name="psum", bufs=3, space="PSUM") as ppool,
            tc.tile_pool(name="outp", bufs=4) as opool,
            tc.tile_pool(name="dram", bufs=1, space="DRAM") as drampool,
        ):
            xjb = cpool.tile([128, 3 * N_RES], F32)
            nc.sync.dma_start(xjb[:], xjb_d[:])
            xi = cpool.tile([128, 4], F32)
            nc.sync.dma_start(xi[:], xi_d[:])
            locol = cpool.tile([128, 1], F32)
            nc.sync.dma_start(locol[:], locol_d[:])
            hicol = cpool.tile([128, 1], F32)
            nc.sync.dma_start(hicol[:], hicol_d[:])
            whi = cpool.tile([128, 1024], BF16)
            nc.sync.dma_start(whi[:], whi_d[:])
            wlo = cpool.tile([128, 1024], FP16)
            nc.sync.dma_start(wlo[:], wlo_d[:])

            # ---- d[i, j] = ((xj0-xi0)^2 + (xj1-xi1)^2) + (xj2-xi2)^2  (exact order)
            sq = []
            for t in range(3):
                s = wpool.tile([128, N_RES], F32, tag="s")
                nc.vector.tensor_scalar(
                    out=s[:],
                    in0=xjb[:, t * N_RES : (t + 1) * N_RES],
                    scalar1=xi[:, t : t + 1],
                    scalar2=None,
                    op0=AluOpType.subtract,
                )
                q = wpool.tile([128, N_RES], F32, tag=f"sq{t}")
                nc.vector.tensor_tensor(out=q[:], in0=s[:], in1=s[:], op=AluOpType.mult)
                sq.append(q)
            d01 = wpool.tile([128, N_RES], F32, tag="d01")
            nc.vector.tensor_tensor(
                out=d01[:], in0=sq[0][:], in1=sq[1][:], op=AluOpType.add
            )
            dfin = dpool.tile([128, N_RES], F32)
            nc.vector.tensor_tensor(
                out=dfin[:], in0=d01[:], in1=sq[2][:], op=AluOpType.add
            )
            # bounce d through DRAM: stride-0 partition replication is only
            # legal on a DRAM-side source AP
            d_dram = drampool.tile([128, N_RES], F32)
            nc.sync.dma_start(d_dram[:], dfin[:])

            for g in range(GROUPS):
                # replicate the 8 d rows of this group across 16 partitions each
                drep = wpool.tile([128, N_RES], F32, tag="drep")
                for il in range(8):
                    nc.sync.dma_start(
                        out=drep[il * 16 : (il + 1) * 16, :],
                        in_=d_dram[g * 8 + il : g * 8 + il + 1, :].to_broadcast(
                            [16, N_RES]
                        ),
                    )
                tlo = wpool.tile([128, N_RES], BF16, tag="tlo")
                nc.vector.tensor_scalar(
                    out=tlo[:],
                    in0=drep[:],
                    scalar1=locol[:, 0:1],
                    scalar2=None,
                    op0=AluOpType.is_gt,
                )
                thi = wpool.tile([128, N_RES], BF16, tag="thi")
                nc.vector.tensor_scalar(
                    out=thi[:],
                    in0=drep[:],
                    scalar1=hicol[:, 0:1],
                    scalar2=None,
                    op0=AluOpType.is_ge,
                )
                ohb = ohpool.tile([128, N_RES], BF16, tag="ohb")
                nc.vector.tensor_tensor(
                    out=ohb[:], in0=tlo[:], in1=thi[:], op=AluOpType.subtract
                )
                ohs = ohpool.tile([128, N_RES], FP16, tag="ohs")
                nc.vector.tensor_scalar(
                    out=ohs[:],
                    in0=ohb[:],
                    scalar1=LO_SCALE,
                    scalar2=None,
                    op0=AluOpType.mult,
                )

                for t in range(JTILES):
                    p0 = ppool.tile([128, 512], F32, tag="p0")
                    p1 = ppool.tile([128, 512], F32, tag="p1")
                    lhs_hi = ohb[:, t * 128 : (t + 1) * 128]
                    lhs_lo = ohs[:, t * 128 : (t + 1) * 128]
                    nc.tensor.matmul(
                        p0[:], lhs_hi, whi[:, 0:512], start=True, stop=False
                    )
                    nc.tensor.matmul(
                        p1[:], lhs_hi, whi[:, 512:1024], start=True, stop=False
                    )
                    nc.tensor.matmul(
                        p0[:], lhs_lo, wlo[:, 0:512], start=False, stop=True
                    )
                    nc.tensor.matmul(
                        p1[:], lhs_lo, wlo[:, 512:1024], start=False, stop=True
                    )
                    ob = opool.tile([128, 1024], F32, tag="ob")
                    nc.vector.tensor_copy(ob[:, 0:512], p0[:])
                    nc.scalar.copy(ob[:, 512:1024], p1[:])
                    nc.sync.dma_start(
                        out=out_d[
                            g * 8 : (g + 1) * 8, t * 128 : (t + 1) * 128, :
                        ].rearrange("il j c -> j il c"),
                        in_=ob[:],
                    )

    nc.compile()
    return nc


def get_program():
    global _PROGRAM
    if _PROGRAM is None:
        _PROGRAM = _build_program()
    return _PROGRAM


def make_in_maps(x: np.ndarray, W: np.ndarray) -> list[dict]:
    x0 = np.asarray(x, dtype=np.float32).reshape(N_RES, 3)
    W = np.asarray(W, dtype=np.float32)

    sq = _sq_bins()
    lo = np.full(16, BIG, dtype=np.float32)
    hi = np.full(16, BIG, dtype=np.float32)
    lo[:NO_BINS] = sq
    hi[: NO_BINS - 1] = sq[1:]
    hi[NO_BINS - 1] = np.float32(INF)
    locol = np.tile(lo, 8)[:, None].astype(np.float32)  # [128, 1]
    hicol = np.tile(hi, 8)[:, None].astype(np.float32)

    wext = np.zeros((16, C_Z), dtype=np.float32)
    wext[:NO_BINS] = W.T  # row k = W[:, k]
    whi_v = wext.astype(ml_dtypes.bfloat16)
    r1 = wext - whi_v.astype(np.float32)
    wlo_v = (r1 * 512.0).astype(np.float16)

    whi_np = np.zeros((8, 16, 8, C_Z), dtype=ml_dtypes.bfloat16)
    wlo_np = np.zeros((8, 16, 8, C_Z), dtype=np.float16)
    for il in range(8):
        whi_np[il, :, il, :] = whi_v
        wlo_np[il, :, il, :] = wlo_v
    whi_np = np.ascontiguousarray(whi_np.reshape(128, 1024))
    wlo_np = np.ascontiguousarray(wlo_np.reshape(128, 1024))

    xjb = np.ascontiguousarray(
        np.broadcast_to(x0.T.reshape(1, 3 * N_RES), (128, 3 * N_RES))
    ).astype(np.float32)

    in_maps = []
    for c in range(N_CORES):
        xi = np.zeros((128, 4), dtype=np.float32)
        xi[:, :3] = x0[c * ROWS_PER_CORE : (c + 1) * ROWS_PER_CORE]
        in_maps.append(
            {
                "xjb": xjb,
                "xi": xi,
                "locol": locol,
                "hicol": hicol,
                "whi": whi_np,
                "wlo": wlo_np,
            }
        )
    return in_maps


def run_on_hw(x, W, trace=False):
    nc = get_program()
    in_maps = make_in_maps(x, W)
    res = bass_utils.run_bass_kernel_spmd(
        nc, in_maps, core_ids=list(range(N_CORES)), trace=trace
    )
    z = np.concatenate([res.results[c]["out"] for c in range(N_CORES)], axis=0)
    return z.reshape(1, N_RES, N_RES, C_Z), res


def kernel(x: np.ndarray, W: np.ndarray, b: np.ndarray) -> np.ndarray:
    z, _ = run_on_hw(x, W)
    b = np.asarray(b, dtype=np.float32)
    if np.any(b != 0.0):
        # reference adds b everywhere (including no-bin pairs); spec fills b with
        # zeros so this never runs in practice, but stay correct if it changes.
        z = z + b.reshape(1, 1, 1, C_Z)
    return z


# revision 8
# speedup vs baseline: 459.2242x; 459.2242x over previous
"""Trainium2 Bass kernel for nn_Ca_Aware_Embedder (histogram distance binning + embed).

z[b, i, j, :] = W[:, bin(d_ij)] + b  where d_ij = ||x_i - x_j||^2 binned into 15
squared-distance buckets (or zeros when d falls below the first edge / on an edge).

Strategy (8 NeuronCores, row-parallel over i, no collectives):
  - d computed per core in [128 i, 1024 j] layout with the reference's exact fp32
    op order ((xj-xi)^2 sums), so binning is bit-exact vs the jax oracle.
  - per group of 8 i-rows: DMA-replicate each d row across 16 partitions so
    partitions = (i_lo, bin_k); two fused DVE compares (d > lo_k) - (d >= hi_k)
    build an exact {0,1} one-hot in bf16.
  - PE produces output tiles [128 j, 8 i x 128 c] via one-hot x block-diagonal W:
    a bf16 pass with W_hi = bf16(W) plus an accumulating fp16 pass with
    (W - W_hi) * 2^9 against one-hot * 2^-9 (exact products; ~2e-7 rel error).
  - PSUM -> SBUF copies split across DVE and ACT, then one 512 KB DMA per tile
    into the output (64 KB contiguous runs in HBM).
"""

import sys

if "/opt/trn_rl_repo" not in sys.path:
    sys.path.insert(0, "/opt/trn_rl_repo")

import numpy as np
import ml_dtypes

import concourse.bass as bass
import concourse.mybir as mybir
import concourse.tile as tile
from concourse import bacc, bass_utils
from concourse.alu_op_type import AluOpType

F32 = mybir.dt.float32
BF16 = mybir.dt.bfloat16
FP16 = mybir.dt.float16

N_RES = 1024
C_Z = 128
NO_BINS = 15
MIN_BIN = 3.25
MAX_BIN = 20.75
INF = 100000000.0
N_CORES = 8
ROWS_PER_CORE = N_RES // N_CORES  # 128
GROUPS = ROWS_PER_CORE // 8  # 16 groups of 8 i-rows
JTILES = N_RES // 128  # 8
BIG = np.float32(3.4028235e38)  # finite sentinel > any d
LO_SCALE = float(2.0 ** -9)

_PROGRAM = None  # (nc, names) cache — build once per process


def _sq_bins() -> np.ndarray:
    """Exact f32 squared bin edges, matching jnp.linspace(...)**2 on this stack."""
    import jax.numpy as jnp

    bins = jnp.linspace(MIN_BIN, MAX_BIN, NO_BINS, dtype=jnp.float32)
    return np.asarray(bins * bins, dtype=np.float32)


def _build_program():
    nc = bacc.Bacc(
        "TRN2",
        target_bir_lowering=False,
        debug=False,
        enable_asserts=False,
        num_devices=N_CORES,
    )

    xjb_d = nc.dram_tensor("xjb", [128, 3 * N_RES], F32, kind="ExternalInput")
    xi_d = nc.dram_tensor("xi", [128, 4], F32, kind="ExternalInput")
    locol_d = nc.dram_tensor("locol", [128, 1], F32, kind="ExternalInput")
    hicol_d = nc.dram_tensor("hicol", [128, 1], F32, kind="ExternalInput")
    whi_d = nc.dram_tensor("whi", [128, 1024], BF16, kind="ExternalInput")
    wlo_d = nc.dram_tensor("wlo", [128, 1024], FP16, kind="ExternalInput")
    out_d = nc.dram_tensor(
        "out", [ROWS_PER_CORE, N_RES, C_Z], F32, kind="ExternalOutput"
    )

    with tile.TileContext(nc) as tc:
        with (
            tc.tile_pool(name="const", bufs=1) as cpool,
            tc.tile_pool(name="dstore", bufs=1) as dpool,
            tc.tile_pool(name="work", bufs=2) as wpool,
            tc.tile_pool(name="ohp", bufs=2) as ohpool,
            tc.tile_pool(name="psum", bufs=3, space="PSUM") as ppool,
            tc.tile_pool(name="outp", bufs=4) as opool,
            tc.tile_pool(name="dram", bufs=1, space="DRAM") as drampool,
        ):
            xjb = cpool.tile([128, 3 * N_RES], F32)
            nc.sync.dma_start(xjb[:], xjb_d[:])
            xi = cpool.tile([128, 4], F32)
            nc.sync.dma_start(xi[:], xi_d[:])
            locol = cpool.tile([128, 1], F32)
            nc.sync.dma_start(locol[:], locol_d[:])
            hicol = cpool.tile([128, 1], F32)
            nc.sync.dma_start(hicol[:], hicol_d[:])
            whi = cpool.tile([128, 1024], BF16)
            nc.sync.dma_start(whi[:], whi_d[:])
            wlo = cpool.tile([128, 1024], FP16)
            nc.sync.dma_start(wlo[:], wlo_d[:])

            # ---- d[i, j] = ((xj0-xi0)^2 + (xj1-xi1)^2) + (xj2-xi2)^2  (exact order)
            sq = []
            for t in range(3):
                s = wpool.tile([128, N_RES], F32, tag="s")
                nc.vector.tensor_scalar(
                    out=s[:],
                    in0=xjb[:, t * N_RES : (t + 1) * N_RES],
                    scalar1=xi[:, t : t + 1],
                    scalar2=None,
                    op0=AluOpType.subtract,
                )
                q = wpool.tile([128, N_RES], F32, tag=f"sq{t}")
                nc.vector.tensor_tensor(out=q[:], in0=s[:], in1=s[:], op=AluOpType.mult)
                sq.append(q)
            d01 = wpool.tile([128, N_RES], F32, tag="d01")
            nc.vector.tensor_tensor(
                out=d01[:], in0=sq[0][:], in1=sq[1][:], op=AluOpType.add
            )
            dfin = dpool.tile([128, N_RES], F32)
            nc.vector.tensor_tensor(
                out=dfin[:], in0=d01[:], in1=sq[2][:], op=AluOpType.add
            )
            # bounce d through DRAM: stride-0 partition replication is only
            # legal on a DRAM-side source AP
            d_dram = drampool.tile([128, N_RES], F32)
            nc.sync.dma_start(d_dram[:], dfin[:])

            for g in range(GROUPS):
                # replicate the 8 d rows of this group across 16 partitions each
                drep = wpool.tile([128, N_RES], F32, tag="drep")
                for il in range(8):
                    nc.sync.dma_start(
                        out=drep[il * 16 : (il + 1) * 16, :],
                        in_=d_dram[g * 8 + il : g * 8 + il + 1, :].to_broadcast(
                            [16, N_RES]
                        ),
                    )
                tlo = wpool.tile([128, N_RES], BF16, tag="tlo")
                nc.vector.tensor_scalar(
                    out=tlo[:],
                    in0=drep[:],
                    scalar1=locol[:, 0:1],
                    scalar2=None,
                    op0=AluOpType.is_gt,
                )
                thi = wpool.tile([128, N_RES], BF16, tag="thi")
                nc.vector.tensor_scalar(
                    out=thi[:],
                    in0=drep[:],
                    scalar1=hicol[:, 0:1],
                    scalar2=None,
                    op0=AluOpType.is_ge,
                )
                ohb = ohpool.tile([128, N_RES], BF16, tag="ohb")
                nc.vector.tensor_tensor(
                    out=ohb[:], in0=tlo[:], in1=thi[:], op=AluOpType.subtract
                )
                ohs = ohpool.tile([128, N_RES], FP16, tag="ohs")
                nc.vector.tensor_scalar(
                    out=ohs[:],
                    in0=ohb[:],
                    scalar1=LO_SCALE,
                    scalar2=None,
                    op0=AluOpType.mult,
                )

                for t in range(JTILES):
                    p0 = ppool.tile([128, 512], F32, tag="p0")
                    p1 = ppool.tile([128, 512], F32, tag="p1")
                    lhs_hi = ohb[:, t * 128 : (t + 1) * 128]
                    lhs_lo = ohs[:, t * 128 : (t + 1) * 128]
                    nc.tensor.matmul(
                        p0[:], lhs_hi, whi[:, 0:512], start=True, stop=False
                    )
                    nc.tensor.matmul(
                        p1[:], lhs_hi, whi[:, 512:1024], start=True, stop=False
                    )
                    nc.tensor.matmul(
                        p0[:], lhs_lo, wlo[:, 0:512], start=False, stop=True
                    )
                    nc.tensor.matmul(
                        p1[:], lhs_lo, wlo[:, 512:1024], start=False, stop=True
                    )
                    ob = opool.tile([128, 1024], F32, tag="ob")
                    nc.vector.tensor_copy(ob[:, 0:512], p0[:])
                    nc.scalar.copy(ob[:, 512:1024], p1[:])
                    nc.sync.dma_start(
                        out=out_d[
                            g * 8 : (g + 1) * 8, t * 128 : (t + 1) * 128, :
                        ].rearrange("il j c -> j il c"),
                        in_=ob[:],
                    )

    nc.compile()
    return nc


def get_program():
    global _PROGRAM
    if _PROGRAM is None:
        _PROGRAM = _build_program()
    return _PROGRAM


def make_in_maps(x: np.ndarray, W: np.ndarray) -> list[dict]:
    x0 = np.asarray(x, dtype=np.float32).reshape(N_RES, 3)
    W = np.asarray(W, dtype=np.float32)

    sq = _sq_bins()
    lo = np.full(16, BIG, dtype=np.float32)
    hi = np.full(16, BIG, dtype=np.float32)
    lo[:NO_BINS] = sq
    hi[: NO_BINS - 1] = sq[1:]
    hi[NO_BINS - 1] = np.float32(INF)
    locol = np.tile(lo, 8)[:, None].astype(np.float32)  # [128, 1]
    hicol = np.tile(hi, 8)[:, None].astype(np.float32)

    wext = np.zeros((16, C_Z), dtype=np.float32)
    wext[:NO_BINS] = W.T  # row k = W[:, k]
    whi_v = wext.astype(ml_dtypes.bfloat16)
    r1 = wext - whi_v.astype(np.float32)
    wlo_v = (r1 * 512.0).astype(np.float16)

    whi_np = np.zeros((8, 16, 8, C_Z), dtype=ml_dtypes.bfloat16)
    wlo_np = np.zeros((8, 16, 8, C_Z), dtype=np.float16)
    for il in range(8):
        whi_np[il, :, il, :] = whi_v
        wlo_np[il, :, il, :] = wlo_v
    whi_np = np.ascontiguousarray(whi_np.reshape(128, 1024))
    wlo_np = np.ascontiguousarray(wlo_np.reshape(128, 1024))

    xjb = np.ascontiguousarray(
        np.broadcast_to(x0.T.reshape(1, 3 * N_RES), (128, 3 * N_RES))
    ).astype(np.float32)

    in_maps = []
    for c in range(N_CORES):
        xi = np.zeros((128, 4), dtype=np.float32)
        xi[:, :3] = x0[c * ROWS_PER_CORE : (c + 1) * ROWS_PER_CORE]
        in_maps.append(
            {
                "xjb": xjb,
                "xi": xi,
                "locol": locol,
                "hicol": hicol,
                "whi": whi_np,
                "wlo": wlo_np,
            }
        )
    return in_maps


_DISPATCH = None  # (sharded_fn, in_names, out_names, out_avals, mesh)


def _build_dispatch():
    import jax
    from jax.sharding import Mesh, PartitionSpec
    from jax.experimental.shard_map import shard_map
    from concourse import bass2jax

    nc = get_program()
    bass2jax.install_neuronx_cc_hook()

    partition_name = (
        nc.partition_id_tensor.name if nc.partition_id_tensor is not None else None
    )
    in_names, out_names, out_avals = [], [], []
    for alloc in nc.m.functions[0].allocations:
        if not isinstance(alloc, mybir.MemoryLocationSet):
            continue
        name = alloc.memorylocations[0].name
        if alloc.kind == "ExternalInput":
            if name != partition_name:
                in_names.append(name)
        elif alloc.kind == "ExternalOutput":
            shape = tuple(alloc.tensor_shape)
            dtype = mybir.dt.np(alloc.dtype)
            out_names.append(name)
            out_avals.append(jax.core.ShapedArray(shape, dtype))
    n_params = len(in_names)
    n_outs = len(out_names)
    all_names = in_names + out_names
    if partition_name is not None:
        all_names = all_names + [partition_name]
    donate = tuple(range(n_params, n_params + n_outs))

    def _body(*args):
        operands = list(args)
        if partition_name is not None:
            operands.append(bass2jax.partition_id_tensor())
        outs = bass2jax._bass_exec_p.bind(
            *operands,
            out_avals=tuple(out_avals),
            in_names=tuple(all_names),
            out_names=tuple(out_names),
            lowering_input_output_aliases=(),
            sim_require_finite=True,
            sim_require_nnan=True,
            nc=nc,
        )
        return tuple(outs)

    devices = jax.devices()[:N_CORES]
    mesh = Mesh(np.asarray(devices), ("core",))
    in_specs = (PartitionSpec("core"),) * (n_params + n_outs)
    out_specs = (PartitionSpec("core"),) * n_outs
    sharded = jax.jit(
        shard_map(
            _body, mesh=mesh, in_specs=in_specs, out_specs=out_specs, check_rep=False
        ),
        donate_argnums=donate,
        keep_unused=True,
    )
    return sharded, in_names, out_names, out_avals, mesh


def get_dispatch():
    global _DISPATCH
    if _DISPATCH is None:
        _DISPATCH = _build_dispatch()
    return _DISPATCH


def _device_args(x, W):
    """Concat per-core inputs along axis 0, put on device; make device zeros."""
    import jax
    import jax.numpy as jnp
    from jax.sharding import NamedSharding, PartitionSpec

    sharded, in_names, out_names, out_avals, mesh = get_dispatch()
    in_maps = make_in_maps(x, W)
    sh = NamedSharding(mesh, PartitionSpec("core"))
    dev_ins = []
    for name in in_names:
        cat = np.concatenate([in_maps[c][name] for c in range(N_CORES)], axis=0)
        dev_ins.append(jax.device_put(cat, sh))
    dev_zeros = [
        jnp.zeros((N_CORES * a.shape[0], *a.shape[1:]), a.dtype, device=sh)
        for a in out_avals
    ]
    return dev_ins, dev_zeros


def run_on_hw(x, W, n_timed: int = 0):
    """Execute on 8 cores. Returns (z, exec_times_s list from timed reruns)."""
    import time as _time
    import jax

    sharded, in_names, out_names, out_avals, mesh = get_dispatch()
    dev_ins, dev_zeros = _device_args(x, W)
    outs = sharded(*dev_ins, *dev_zeros)
    jax.block_until_ready(outs)
    times = []
    for _ in range(n_timed):
        t0 = _time.perf_counter()
        outs = sharded(*dev_ins, *outs)  # re-donate previous outputs
        jax.block_until_ready(outs)
        times.append(_time.perf_counter() - t0)
    out_arr = np.asarray(outs[0])  # [8*128, 1024, 128]
    z = out_arr.reshape(1, N_RES, N_RES, C_Z)
    return z, times


def kernel(x: np.ndarray, W: np.ndarray, b: np.ndarray) -> np.ndarray:
    z, _ = run_on_hw(x, W, n_timed=0)
    b = np.asarray(b, dtype=np.float32)
    if np.any(b != 0.0):
        # reference adds b everywhere (including no-bin pairs); spec fills b with
        # zeros so this never runs in practice, but stay correct if it changes.
        z = z + b.reshape(1, 1, 1, C_Z)
    return z
